# revision 1
# baseline (speedup 1.0000x reference)
"""Trainium2 Bass kernel for nn_DagnabbitAutoEncoder (gnn_message_passing).

Self-contained: kernel(**inputs) takes FULL inputs, returns FULL [B,N,D]
output. Strategy: data-parallel over graphs across 8 NeuronCores; on the
host, the DAG scan is converted into ~24 level-wavefronts (level =
longest-path depth); each level is processed on-device as one batched
dma_gather (parent embeddings) -> per-type MLP (PE/ACT/DVE) -> one
dma_scatter_add (into the pre-zeroed output buffer, so += == write).
"""

B_, N_, R_, D_, K_, T_, M_ = 256, 2048, 64, 64, 2, 8, 8

"""DAG autoencoder Trainium kernel v2: ANT dma_gather/dma_scatter_add based.

Per core, the 32 graphs are split into 3 units of [11, 11, 10] graphs. Each
unit owns a DRAM output tensor of size_u*N + 1 rows (last row = scratch for
pad scatters; dropped on host). Row of node i of unit-local graph b:
b*N + i (int16-addressable: max 11*2048 = 22528).

Schedule: per (unit, level): nodes grouped by type; each (unit, level, type)
group padded across cores to a multiple of 128 slots. Slot s of a
(unit, level) maps to (partition p = s%128, chunk c = s//128); chunks are
type-pure.

dma_gather fetch order per (unit, level): parent0 of slots 0..S-1, then
parent1 of slots 0..S-1 (S multiple of 128), so the x tile [128, 2C, 64]
holds parent0 at block c, parent1 at block C+c for the item at (p, c).
Gather pads fetch row 0 (valid, data unused). Scatter pads target the
scratch row.

Compute per chunk (type t): PE transpose x -> xT (feat-major, PSUM), DVE
copy to SBUF, MM1 (lhsT=W1[t]) -> hT PSUM, ACT gelu+b1[t] (per-partition
bias) -> SBUF, MM2 (lhsT=hT, rhs=W2[t]) -> item-major out PSUM, DVE +b2
broadcast tile -> osb. Per-level dma_scatter_add writes rows (output buffer
is pre-zeroed, so += is a plain write).
"""

import numpy as np

UNIT_SIZES = None  # set by plan_units


def plan_units(BL):
    """Split BL graphs into units of <=15 graphs (int16 row space with
    scratch: 15*2048+1 = 30721 <= 32767)."""
    sizes = []
    rem = BL
    nu = max(1, -(-BL // 11))
    base = BL // nu
    extra = BL - base * nu
    for u in range(nu):
        sizes.append(base + (1 if u < extra else 0))
    assert sum(sizes) == BL and all(s * 2048 + 1 <= 32767 for s in sizes)
    return sizes


def compute_levels(idx, R, N):
    B = idx.shape[0]
    lvl = np.zeros((B, N), np.int32)
    ar = np.arange(B)
    for i in range(R, N):
        lvl[:, i] = 1 + lvl[ar[:, None], idx[:, i, :]].max(axis=1)
    return lvl


def _wrap16(vals, ncols):
    """vals [n] -> [128, ncols] int16 in the wrapped-16 + replicated layout:
    fetch k at (k%16, k//16), rows replicated to all 8 stripes of 16."""
    n = len(vals)
    arr = np.zeros((16, ncols), np.int16)
    k = np.arange(n)
    arr[k % 16, k // 16] = vals
    return np.tile(arr, (8, 1))


def build_schedule(idx, types, B, N, R, T, M):
    BL = B // M
    unit_sizes = plan_units(BL)
    NU = len(unit_sizes)
    ubase = np.concatenate([[0], np.cumsum(unit_sizes)])  # graph offset per unit

    lvl = compute_levels(idx, R, N)
    L = int(lvl[:, R:].max())
    types_np = np.asarray(types)

    # counts per (core, unit, level, type) -> max over cores
    cnt = np.zeros((M, NU, L + 1, T), np.int64)
    for m in range(M):
        for u in range(NU):
            g0 = m * BL + ubase[u]
            g1 = g0 + unit_sizes[u]
            np.add.at(
                cnt[m, u],
                (lvl[g0:g1, R:].ravel(), types_np[g0:g1, R:].ravel()),
                1,
            )
    maxcnt = cnt.max(axis=0)  # [NU, L+1, T]
    chunks = -(-maxcnt // 128)
    chunks[:, 0, :] = 0
    C_ul = chunks.sum(axis=2).astype(int)  # [NU, L+1]

    # idx16 column layout: per level, per unit: [gather 2S/16][scatter S/16]
    gcol = np.zeros((NU, L + 1), int)
    scol = np.zeros((NU, L + 1), int)
    w = 0
    for l in range(1, L + 1):
        for u in range(NU):
            S = int(C_ul[u, l]) * 128
            gcol[u, l] = w
            w += (2 * S) // 16
            scol[u, l] = w
            w += S // 16
    W16 = max(w, 1)

    idx16_per_core = []
    for m in range(M):
        arr = np.zeros((128, W16), np.int16)
        for l in range(1, L + 1):
            for u in range(NU):
                S = int(C_ul[u, l]) * 128
                if S == 0:
                    continue
                g0 = m * BL + ubase[u]
                gvals = np.zeros(2 * S, np.int64)  # pads -> row 0
                svals = np.full(S, unit_sizes[u] * N, np.int64)  # pads -> scratch
                cbase = 0
                for t in range(T):
                    nch = int(chunks[u, l, t])
                    if nch == 0:
                        continue
                    bb, ii = np.nonzero(
                        (lvl[g0 : g0 + unit_sizes[u], R:] == l)
                        & (types_np[g0 : g0 + unit_sizes[u], R:] == t)
                    )
                    ii = ii + R
                    s = cbase * 128 + np.arange(len(bb))
                    # chunk-interleaved fetch order: chunk c's parent0 at
                    # fetches c*256..+127, parent1 at c*256+128..+255
                    pos0 = (s // 128) * 256 + (s % 128)
                    gvals[pos0] = bb * N + idx[g0 + bb, ii, 0]
                    gvals[pos0 + 128] = bb * N + idx[g0 + bb, ii, 1]
                    svals[s] = bb * N + ii
                    cbase += nch
                arr[:, gcol[u, l] : gcol[u, l] + (2 * S) // 16] = _wrap16(
                    gvals, (2 * S) // 16
                )
                arr[:, scol[u, l] : scol[u, l] + S // 16] = _wrap16(svals, S // 16)
        idx16_per_core.append(arr)

    chunk_types = [
        [
            [t for t in range(T) for _ in range(int(chunks[u, l, t]))]
            for l in range(L + 1)
        ]
        for u in range(NU)
    ]

    return dict(
        L=L,
        BL=BL,
        NU=NU,
        unit_sizes=unit_sizes,
        ubase=ubase,
        C_ul=C_ul,
        chunk_types=chunk_types,
        gcol=gcol,
        scol=scol,
        W16=W16,
        idx16_per_core=idx16_per_core,
    )


def build_inputs(root_embeddings, W1, b1, W2, b2, sched, N, R, D, T, M):
    BL = sched["BL"]
    w1 = np.ascontiguousarray(
        np.transpose(np.asarray(W1), (1, 0, 2)).reshape(2 * D, T * 2 * D)
    )
    w2 = np.ascontiguousarray(
        np.transpose(np.asarray(W2), (1, 0, 2)).reshape(2 * D, T * D)
    )
    b1a = np.ascontiguousarray(np.asarray(b1).T)
    b2f = np.broadcast_to(np.asarray(b2).reshape(1, T * D), (128, T * D)).copy()

    roots_np = np.asarray(root_embeddings)
    in_maps = []
    for m in range(M):
        R_ = roots_np.shape[1]
        rts = np.ascontiguousarray(
            roots_np[m * BL : (m + 1) * BL].transpose(1, 0, 2).reshape(R_, -1)
        )
        in_maps.append(
            dict(
                roots=rts.astype(np.float32),
                w1=w1.astype(np.float32),
                w2=w2.astype(np.float32),
                b1=b1a.astype(np.float32),
                b2f=b2f.astype(np.float32),
                idx16=sched["idx16_per_core"][m],
            )
        )
    return in_maps


def _cap_waits(nc, max_waits=1):
    """Walrus per-instruction sync-wait slots are tiny (1 for DMA/LDW/nop
    encodings observed); move excess waits onto preceding same-engine
    single-wait nops (identical ordering: same engine queue)."""
    import concourse.mybir as mb

    k = 0
    for f in nc.m.functions:
        for bb in f.blocks:
            out = []
            for ins in bb.instructions:
                si = getattr(ins, "sync_info", None)
                if si is not None and si.on_wait and len(si.on_wait) > max_waits:
                    waits = list(si.on_wait)
                    keep = waits[:max_waits]
                    for w in waits[max_waits:]:
                        nop = mb.InstNoOp(name=f"waitnop_{k}", ins=[], outs=[])
                        k += 1
                        nop.engine = ins.engine
                        nop.sync_info = mb.SyncInfo(on_wait=[w], on_update=[])
                        out.append(nop)
                    ins.sync_info = mb.SyncInfo(
                        on_wait=keep, on_update=list(si.on_update or [])
                    )
                out.append(ins)
            bb.instructions = out
    return k


def build_program(sched, N, R, D, T, split_waits=True):
    import concourse.bass as bass
    import concourse.mybir as mybir
    from concourse import tile
    from concourse.masks import make_identity
    from concourse.tile_rust import add_dep_helper

    BL = sched["BL"]
    L = sched["L"]
    NU = sched["NU"]
    unit_sizes = sched["unit_sizes"]
    ubase = sched["ubase"]
    C_ul = sched["C_ul"]
    chunk_types = sched["chunk_types"]
    gcol = sched["gcol"]
    scol = sched["scol"]
    W16 = sched["W16"]
    f32 = mybir.dt.float32
    i16 = mybir.dt.int16

    nc = bass.Bass()
    roots = nc.declare_dram_parameter("roots", [R, BL * D], f32, isOutput=False)
    w1 = nc.declare_dram_parameter("w1", [2 * D, T * 2 * D], f32, isOutput=False)
    w2 = nc.declare_dram_parameter("w2", [2 * D, T * D], f32, isOutput=False)
    b1 = nc.declare_dram_parameter("b1", [2 * D, T], f32, isOutput=False)
    b2f = nc.declare_dram_parameter("b2f", [128, T * D], f32, isOutput=False)
    idx16 = nc.declare_dram_parameter("idx16", [128, W16], i16, isOutput=False)
    outs = [
        nc.declare_dram_parameter(
            f"out{u}", [unit_sizes[u] * N + 1, D], f32, isOutput=True
        )
        for u in range(NU)
    ]

    Cmax = [int(C_ul[u, 1:].max()) for u in range(NU)]

    with tile.TileContext(nc) as tc:
        with (
            tc.tile_pool(name="const", bufs=1) as constp,
            tc.tile_pool(name="lvl", bufs=2) as lvlp,
            tc.tile_pool(name="work", bufs=4) as workp,
            tc.tile_pool(name="ps", bufs=2, space="PSUM") as psp,
        ):
            from concourse import library_config

            nc.gpsimd.load_library(library_config.mlp)
            ident = constp.tile([128, 128], f32)
            make_identity(nc, ident[:])
            w1sb = constp.tile([2 * D, T * 2 * D], f32)
            nc.sync.dma_start(out=w1sb[:], in_=w1[:])
            w2sb = constp.tile([2 * D, T * D], f32)
            nc.sync.dma_start(out=w2sb[:], in_=w2[:])
            b1sb = constp.tile([2 * D, T], f32)
            nc.sync.dma_start(out=b1sb[:], in_=b1[:])
            b2fsb = constp.tile([128, T * D], f32)
            nc.sync.dma_start(out=b2fsb[:], in_=b2f[:])
            idxsb = constp.tile([128, W16], i16)
            nc.sync.dma_start(out=idxsb[:], in_=idx16[:])

            # roots: [R, BL, D] via SBUF, then per-unit strided store
            rt_sb = constp.tile([R, BL, D], f32)
            nc.sync.dma_start(
                out=rt_sb[:], in_=roots[:].rearrange("r (g d) -> r g d", g=BL)
            )
            root_inits = []
            for u in range(NU):
                gu = unit_sizes[u]
                ri = nc.sync.dma_start(
                    out=outs[u][0 : gu * N, :].rearrange(
                        "(g n) d -> n g d", g=gu
                    )[0:R],
                    in_=rt_sb[:, ubase[u] : ubase[u] + gu, :],
                )
                root_inits.append(ri)

            # one register per distinct count value: the Q7 ucode may read the
            # count register asynchronously, so never rewrite a live register
            _regcache = {}

            def creg_for(v):
                if v not in _regcache:
                    _regcache[v] = nc.gpsimd.to_reg(v)
                return _regcache[v]

            prev_scatter = [None] * NU
            for l in range(1, L + 1):
                xs = {}
                osbs = {}
                gathers = {}
                for u in range(NU):
                    C = int(C_ul[u, l])
                    if C == 0:
                        continue
                    S = C * 128
                    x = lvlp.tile([128, 2 * Cmax[u], D], f32, tag=f"x{u}")
                    xs[u] = (x, C)
                    g = nc.gpsimd.dma_gather(
                        out_ap=x[:, 0 : 2 * C, :],
                        in_ap=outs[u][:],
                        idxs_ap=idxsb[:, gcol[u, l] : gcol[u, l] + (2 * S) // 16],
                        num_idxs=2 * S,
                        num_idxs_reg=creg_for(2 * S),
                        elem_size=D,
                        single_packet=False,
                    )
                    dep = prev_scatter[u] if prev_scatter[u] is not None else root_inits[u]
                    add_dep_helper(g.ins, dep.ins, sync=True, reason="lvl order")
                    gathers[u] = g

                for u in range(NU):
                    if u not in xs:
                        continue
                    x, C = xs[u]
                    osb = lvlp.tile([128, Cmax[u], D], f32, tag=f"o{u}")
                    osbs[u] = (osb, C)
                    for c in range(C):
                        t = chunk_types[u][l][c]
                        xT_ps = psp.tile([128, 128], f32, tag="xT")
                        x_ch = x[:, 2 * c : 2 * c + 2, :].rearrange(
                            "p a e -> p (a e)"
                        )
                        nc.tensor.transpose(xT_ps[:], x_ch, ident[:])
                        xT = workp.tile([128, 128], f32, tag="xTs")
                        nc.vector.tensor_copy(xT[:], xT_ps[:])
                        hT_ps = psp.tile([128, 128], f32, tag="hT")
                        nc.tensor.matmul(
                            hT_ps[:],
                            lhsT=w1sb[:, t * 2 * D : (t + 1) * 2 * D],
                            rhs=xT[:],
                            start=True,
                            stop=True,
                        )
                        hT = workp.tile([128, 128], f32, tag="hTs")
                        nc.scalar.activation(
                            hT[:],
                            hT_ps[:],
                            mybir.ActivationFunctionType.Gelu,
                            bias=b1sb[:, t : t + 1],
                        )
                        o_ps = psp.tile([128, D], f32, tag="o")
                        nc.tensor.matmul(
                            o_ps[:],
                            lhsT=hT[:],
                            rhs=w2sb[:, t * D : (t + 1) * D],
                            start=True,
                            stop=True,
                        )
                        nc.vector.tensor_tensor(
                            out=osb[:, c, :],
                            in0=o_ps[:],
                            in1=b2fsb[:, t * D : (t + 1) * D],
                            op=mybir.AluOpType.add,
                        )

                for u in range(NU):
                    if u not in osbs:
                        continue
                    osb, C = osbs[u]
                    S = C * 128
                    s = nc.gpsimd.dma_scatter_add(
                        out_ap=outs[u][:],
                        in_ap=osb[:, 0:C, :],
                        idxs_ap=idxsb[:, scol[u, l] : scol[u, l] + S // 16],
                        num_idxs=S,
                        num_idxs_reg=creg_for(S),
                        elem_size=D,
                        single_packet=False,
                    )
                    add_dep_helper(s.ins, gathers[u].ins, sync=True, reason="war")
                    prev_scatter[u] = s

    from concourse.library_overlay import lower_extended_insts

    lower_extended_insts(nc)
    if split_waits:
        _cap_waits(nc)
    return nc


def assemble_output(results, sched, N, D, M):
    """results: list per core of dict out{u} -> np.ndarray."""
    NU = sched["NU"]
    unit_sizes = sched["unit_sizes"]
    parts = []
    for m in range(M):
        gs = [
            results[m][f"out{u}"][: unit_sizes[u] * N].reshape(unit_sizes[u], N, D)
            for u in range(NU)
        ]
        parts.append(np.concatenate(gs, axis=0))
    return np.concatenate(parts, axis=0)


def kernel(**inputs):
    import numpy as np

    root_embeddings = np.asarray(inputs["root_embeddings"], np.float32)
    W1 = np.asarray(inputs["W1"], np.float32)
    b1 = np.asarray(inputs["b1"], np.float32)
    W2 = np.asarray(inputs["W2"], np.float32)
    b2 = np.asarray(inputs["b2"], np.float32)
    idx = np.asarray(inputs["node_inputs_indices"], np.int32)
    types = np.asarray(inputs["node_types"], np.int32)

    B, N, R, D, T, M = B_, N_, R_, D_, T_, M_
    sched = build_schedule(idx, types, B, N, R, T, M)
    in_maps = build_inputs(root_embeddings, W1, b1, W2, b2, sched, N, R, D, T, M)
    nc = build_program(sched, N, R, D, T)

    from concourse.bass_utils import run_bass_kernel_spmd

    res = run_bass_kernel_spmd(nc, in_maps, core_ids=list(range(M)))
    out = assemble_output(res.results, sched, N, D, M)
    return out.astype(np.float32)



# revision 9
# speedup vs baseline: 1.6098x; 1.6098x over previous
"""Trainium2 Bass kernel for nn_DagnabbitAutoEncoder (gnn_message_passing).

Self-contained: kernel(**inputs) takes FULL inputs, returns FULL [B,N,D]
output. Strategy: data-parallel over graphs across 8 NeuronCores; on the
host, the DAG scan is converted into ~24 level-wavefronts (level =
longest-path depth); each level is processed on-device as one batched
dma_gather (parent embeddings) -> per-type MLP (PE/ACT/DVE) -> one
dma_scatter_add (into the pre-zeroed output buffer, so += == write).
"""

B_, N_, R_, D_, K_, T_, M_ = 256, 2048, 64, 64, 2, 8, 8

"""DAG autoencoder Trainium kernel v2: ANT dma_gather/dma_scatter_add based.

Per core, the 32 graphs are split into 3 units of [11, 11, 10] graphs. Each
unit owns a DRAM output tensor of size_u*N + 1 rows (last row = scratch for
pad scatters; dropped on host). Row of node i of unit-local graph b:
b*N + i (int16-addressable: max 11*2048 = 22528).

Schedule: per (unit, level): nodes grouped by type; each (unit, level, type)
group padded across cores to a multiple of 128 slots. Slot s of a
(unit, level) maps to (partition p = s%128, chunk c = s//128); chunks are
type-pure.

dma_gather fetch order per (unit, level): parent0 of slots 0..S-1, then
parent1 of slots 0..S-1 (S multiple of 128), so the x tile [128, 2C, 64]
holds parent0 at block c, parent1 at block C+c for the item at (p, c).
Gather pads fetch row 0 (valid, data unused). Scatter pads target the
scratch row.

Compute per chunk (type t): PE transpose x -> xT (feat-major, PSUM), DVE
copy to SBUF, MM1 (lhsT=W1[t]) -> hT PSUM, ACT gelu+b1[t] (per-partition
bias) -> SBUF, MM2 (lhsT=hT, rhs=W2[t]) -> item-major out PSUM, DVE +b2
broadcast tile -> osb. Per-level dma_scatter_add writes rows (output buffer
is pre-zeroed, so += is a plain write).
"""

import numpy as np

UNIT_SIZES = None  # set by plan_units


def plan_units(BL):
    """Split BL graphs into units of <=15 graphs (int16 row space with
    scratch: 15*2048+1 = 30721 <= 32767)."""
    sizes = []
    rem = BL
    nu = max(1, -(-BL // 11))
    base = BL // nu
    extra = BL - base * nu
    for u in range(nu):
        sizes.append(base + (1 if u < extra else 0))
    assert sum(sizes) == BL and all(s * 2048 + 1 <= 32767 for s in sizes)
    return sizes


def compute_levels(idx, R, N):
    B = idx.shape[0]
    lvl = np.zeros((B, N), np.int32)
    ar = np.arange(B)
    for i in range(R, N):
        lvl[:, i] = 1 + lvl[ar[:, None], idx[:, i, :]].max(axis=1)
    return lvl


def _wrap16(vals, ncols):
    """vals [n] -> [128, ncols] int16 in the wrapped-16 + replicated layout:
    fetch k at (k%16, k//16), rows replicated to all 8 stripes of 16."""
    n = len(vals)
    arr = np.zeros((16, ncols), np.int16)
    k = np.arange(n)
    arr[k % 16, k // 16] = vals
    return np.tile(arr, (8, 1))


def build_schedule(idx, types, B, N, R, T, M):
    BL = B // M
    unit_sizes = plan_units(BL)
    NU = len(unit_sizes)
    ubase = np.concatenate([[0], np.cumsum(unit_sizes)])  # graph offset per unit

    lvl = compute_levels(idx, R, N)
    L = int(lvl[:, R:].max())
    types_np = np.asarray(types)

    # counts per (core, unit, level, type) -> max over cores
    cnt = np.zeros((M, NU, L + 1, T), np.int64)
    for m in range(M):
        for u in range(NU):
            g0 = m * BL + ubase[u]
            g1 = g0 + unit_sizes[u]
            np.add.at(
                cnt[m, u],
                (lvl[g0:g1, R:].ravel(), types_np[g0:g1, R:].ravel()),
                1,
            )
    maxcnt = cnt.max(axis=0)  # [NU, L+1, T]
    chunks = -(-maxcnt // 128)
    chunks[:, 0, :] = 0
    C_ul = chunks.sum(axis=2).astype(int)  # [NU, L+1]

    # idx16 column layout: per level, per unit: [gather 2S/16][scatter S/16]
    gcol = np.zeros((NU, L + 1), int)
    scol = np.zeros((NU, L + 1), int)
    w = 0
    for l in range(1, L + 1):
        for u in range(NU):
            S = int(C_ul[u, l]) * 128
            gcol[u, l] = w
            w += (2 * S) // 16
            scol[u, l] = w
            w += S // 16
    W16 = max(w, 1)

    idx16_per_core = []
    for m in range(M):
        arr = np.zeros((128, W16), np.int16)
        for l in range(1, L + 1):
            for u in range(NU):
                S = int(C_ul[u, l]) * 128
                if S == 0:
                    continue
                g0 = m * BL + ubase[u]
                gvals = np.zeros(2 * S, np.int64)  # pads -> row 0
                svals = np.full(S, unit_sizes[u] * N, np.int64)  # pads -> scratch
                cbase = 0
                for t in range(T):
                    nch = int(chunks[u, l, t])
                    if nch == 0:
                        continue
                    bb, ii = np.nonzero(
                        (lvl[g0 : g0 + unit_sizes[u], R:] == l)
                        & (types_np[g0 : g0 + unit_sizes[u], R:] == t)
                    )
                    ii = ii + R
                    s = cbase * 128 + np.arange(len(bb))
                    # chunk-interleaved fetch order: chunk c's parent0 at
                    # fetches c*256..+127, parent1 at c*256+128..+255
                    pos0 = (s // 128) * 256 + (s % 128)
                    gvals[pos0] = bb * N + idx[g0 + bb, ii, 0]
                    gvals[pos0 + 128] = bb * N + idx[g0 + bb, ii, 1]
                    svals[s] = bb * N + ii
                    cbase += nch
                arr[:, gcol[u, l] : gcol[u, l] + (2 * S) // 16] = _wrap16(
                    gvals, (2 * S) // 16
                )
                arr[:, scol[u, l] : scol[u, l] + S // 16] = _wrap16(svals, S // 16)
        idx16_per_core.append(arr)

    chunk_types = [
        [
            [t for t in range(T) for _ in range(int(chunks[u, l, t]))]
            for l in range(L + 1)
        ]
        for u in range(NU)
    ]

    return dict(
        L=L,
        BL=BL,
        NU=NU,
        unit_sizes=unit_sizes,
        ubase=ubase,
        C_ul=C_ul,
        chunk_types=chunk_types,
        gcol=gcol,
        scol=scol,
        W16=W16,
        idx16_per_core=idx16_per_core,
    )


def build_inputs(root_embeddings, W1, b1, W2, b2, sched, N, R, D, T, M):
    import ml_dtypes

    BL = sched["BL"]
    w1 = np.ascontiguousarray(
        np.transpose(np.asarray(W1), (1, 0, 2)).reshape(2 * D, T * 2 * D)
    ).astype(ml_dtypes.bfloat16)
    w2 = np.ascontiguousarray(
        np.transpose(np.asarray(W2), (1, 0, 2)).reshape(2 * D, T * D)
    ).astype(ml_dtypes.bfloat16)
    b1a = np.ascontiguousarray(np.asarray(b1).T)
    b2f = np.broadcast_to(np.asarray(b2).reshape(1, T * D), (128, T * D)).copy()

    roots_np = np.asarray(root_embeddings)
    in_maps = []
    for m in range(M):
        R_ = roots_np.shape[1]
        rts = np.ascontiguousarray(
            roots_np[m * BL : (m + 1) * BL].transpose(1, 0, 2).reshape(R_, -1)
        )
        in_maps.append(
            dict(
                roots=rts.astype(np.float32),
                w1=w1,
                w2=w2,
                b1=b1a.astype(np.float32),
                b2f=b2f.astype(np.float32),
                idx16=sched["idx16_per_core"][m],
            )
        )
    return in_maps


def _cap_waits(nc, max_waits=1):
    """Walrus per-instruction sync-wait slots are tiny (1 for DMA/LDW/nop
    encodings observed); move excess waits onto preceding same-engine
    single-wait nops (identical ordering: same engine queue)."""
    import concourse.mybir as mb

    k = 0
    for f in nc.m.functions:
        for bb in f.blocks:
            out = []
            for ins in bb.instructions:
                si = getattr(ins, "sync_info", None)
                if si is not None and si.on_wait and len(si.on_wait) > max_waits:
                    waits = list(si.on_wait)
                    keep = waits[:max_waits]
                    for w in waits[max_waits:]:
                        nop = mb.InstNoOp(name=f"waitnop_{k}", ins=[], outs=[])
                        k += 1
                        nop.engine = ins.engine
                        nop.sync_info = mb.SyncInfo(on_wait=[w], on_update=[])
                        out.append(nop)
                    ins.sync_info = mb.SyncInfo(
                        on_wait=keep, on_update=list(si.on_update or [])
                    )
                out.append(ins)
            bb.instructions = out
    return k


def build_program(sched, N, R, D, T, split_waits=True, zero_b1=True, zero_b2=True):
    import concourse.bass as bass
    import concourse.mybir as mybir
    from concourse import tile
    from concourse.masks import make_identity
    from concourse.tile_rust import add_dep_helper

    BL = sched["BL"]
    L = sched["L"]
    NU = sched["NU"]
    unit_sizes = sched["unit_sizes"]
    ubase = sched["ubase"]
    C_ul = sched["C_ul"]
    chunk_types = sched["chunk_types"]
    gcol = sched["gcol"]
    scol = sched["scol"]
    W16 = sched["W16"]
    f32 = mybir.dt.float32
    bf16 = mybir.dt.bfloat16
    i16 = mybir.dt.int16

    nc = bass.Bass()
    roots = nc.declare_dram_parameter("roots", [R, BL * D], f32, isOutput=False)
    w1 = nc.declare_dram_parameter("w1", [2 * D, T * 2 * D], bf16, isOutput=False)
    w2 = nc.declare_dram_parameter("w2", [2 * D, T * D], bf16, isOutput=False)
    b1 = nc.declare_dram_parameter("b1", [2 * D, T], f32, isOutput=False)
    b2f = nc.declare_dram_parameter("b2f", [128, T * D], f32, isOutput=False)
    idx16 = nc.declare_dram_parameter("idx16", [128, W16], i16, isOutput=False)
    outs = [
        nc.declare_dram_parameter(
            f"out{u}", [unit_sizes[u] * N + 1, D], f32, isOutput=True
        )
        for u in range(NU)
    ]

    Cmax = [int(C_ul[u, 1:].max()) for u in range(NU)]

    with tile.TileContext(nc) as tc:
        with (
            tc.tile_pool(name="const", bufs=1) as constp,
            tc.tile_pool(name="lvl", bufs=2) as lvlp,
            tc.tile_pool(name="work", bufs=4) as workp,
            tc.tile_pool(name="ps", bufs=2, space="PSUM") as psp,
        ):
            from concourse import library_config

            nc.gpsimd.load_library(library_config.mlp)
            ident = constp.tile([128, 128], f32)
            make_identity(nc, ident[:])
            w1sb = constp.tile([2 * D, T * 2 * D], bf16)
            nc.sync.dma_start(out=w1sb[:], in_=w1[:])
            w2sb = constp.tile([2 * D, T * D], bf16)
            nc.sync.dma_start(out=w2sb[:], in_=w2[:])
            b1sb = constp.tile([2 * D, T], f32)
            nc.sync.dma_start(out=b1sb[:], in_=b1[:])
            b2fsb = constp.tile([128, T * D], f32)
            nc.sync.dma_start(out=b2fsb[:], in_=b2f[:])
            idxsb = constp.tile([128, W16], i16)
            nc.sync.dma_start(out=idxsb[:], in_=idx16[:])

            # roots: [R, BL, D] via SBUF, then per-unit strided store
            rt_sb = constp.tile([R, BL, D], f32)
            nc.sync.dma_start(
                out=rt_sb[:], in_=roots[:].rearrange("r (g d) -> r g d", g=BL)
            )
            root_inits = []
            for u in range(NU):
                gu = unit_sizes[u]
                ri = nc.sync.dma_start(
                    out=outs[u][0 : gu * N, :].rearrange(
                        "(g n) d -> n g d", g=gu
                    )[0:R],
                    in_=rt_sb[:, ubase[u] : ubase[u] + gu, :],
                )
                root_inits.append(ri)

            # one register per distinct count value: the Q7 ucode may read the
            # count register asynchronously, so never rewrite a live register
            _regcache = {}

            def creg_for(v):
                if v not in _regcache:
                    _regcache[v] = nc.gpsimd.to_reg(v)
                return _regcache[v]

            prev_scatter = [None] * NU
            _drain_rr = [0]
            for l in range(1, L + 1):
                xs = {}
                osbs = {}
                gathers = {}
                for u in range(NU):
                    C = int(C_ul[u, l])
                    if C == 0:
                        continue
                    S = C * 128
                    x = lvlp.tile([128, 2 * Cmax[u], D], f32, tag=f"x{u}")
                    xs[u] = (x, C)
                    g = nc.gpsimd.dma_gather(
                        out_ap=x[:, 0 : 2 * C, :],
                        in_ap=outs[u][:],
                        idxs_ap=idxsb[:, gcol[u, l] : gcol[u, l] + (2 * S) // 16],
                        num_idxs=2 * S,
                        num_idxs_reg=creg_for(2 * S),
                        elem_size=D,
                        single_packet=False,
                    )
                    dep = prev_scatter[u] if prev_scatter[u] is not None else root_inits[u]
                    add_dep_helper(g.ins, dep.ins, sync=True, reason="lvl order")
                    gathers[u] = g

                for u in range(NU):
                    if u not in xs:
                        continue
                    x, C = xs[u]
                    osb = lvlp.tile([128, Cmax[u], D], f32, tag=f"o{u}")
                    osbs[u] = (osb, C)
                    G = 4
                    for c0 in range(0, C, G):
                        gn = min(G, C - c0)
                        # 1) PE transposes into one shared PSUM tile
                        xT_ps = psp.tile([128, G * 128], f32, tag="xT")
                        for g in range(gn):
                            c = c0 + g
                            x_ch = x[:, 2 * c : 2 * c + 2, :].rearrange(
                                "p a e -> p (a e)"
                            )
                            nc.tensor.transpose(
                                xT_ps[:, g * 128 : (g + 1) * 128], x_ch, ident[:]
                            )
                        # 2) one batched drain fp32->bf16 (cycle DVE/DVE/ACT)
                        xT = workp.tile([128, G * 128], bf16, tag="xTs")
                        _dr = _drain_rr[0]
                        _drain_rr[0] += 1
                        if _dr % 3 == 2:
                            nc.scalar.activation(
                                xT[:, : gn * 128],
                                xT_ps[:, : gn * 128],
                                mybir.ActivationFunctionType.Identity,
                                bias=b1sb[:, 0:1],
                            )
                        else:
                            nc.vector.tensor_copy(
                                xT[:, : gn * 128], xT_ps[:, : gn * 128]
                            )
                        # 3) per-chunk MM1 (bf16) into shared PSUM
                        hT_ps = psp.tile([128, G * 128], f32, tag="hT")
                        for g in range(gn):
                            t = chunk_types[u][l][c0 + g]
                            nc.tensor.matmul(
                                hT_ps[:, g * 128 : (g + 1) * 128],
                                lhsT=w1sb[:, t * 2 * D : (t + 1) * 2 * D],
                                rhs=xT[:, g * 128 : (g + 1) * 128],
                                start=True,
                                stop=True,
                            )
                        # 4) gelu: batched if b1 == 0, else per chunk
                        hT = workp.tile([128, G * 128], bf16, tag="hTs")
                        if zero_b1:
                            nc.scalar.activation(
                                hT[:, : gn * 128],
                                hT_ps[:, : gn * 128],
                                mybir.ActivationFunctionType.Gelu,
                                bias=b1sb[:, 0:1],
                            )
                        else:
                            for g in range(gn):
                                t = chunk_types[u][l][c0 + g]
                                nc.scalar.activation(
                                    hT[:, g * 128 : (g + 1) * 128],
                                    hT_ps[:, g * 128 : (g + 1) * 128],
                                    mybir.ActivationFunctionType.Gelu,
                                    bias=b1sb[:, t : t + 1],
                                )
                        # 5) per-chunk MM2 (bf16) into shared PSUM
                        o_ps = psp.tile([128, G * D], f32, tag="o")
                        for g in range(gn):
                            t = chunk_types[u][l][c0 + g]
                            nc.tensor.matmul(
                                o_ps[:, g * D : (g + 1) * D],
                                lhsT=hT[:, g * 128 : (g + 1) * 128],
                                rhs=w2sb[:, t * D : (t + 1) * D],
                                start=True,
                                stop=True,
                            )
                        # 6) output drain: batched copy if b2 == 0
                        if zero_b2:
                            nc.vector.tensor_copy(
                                osb[:, c0 : c0 + gn, :].rearrange(
                                    "p g e -> p (g e)"
                                ),
                                o_ps[:, : gn * D],
                            )
                        else:
                            for g in range(gn):
                                t = chunk_types[u][l][c0 + g]
                                nc.vector.tensor_tensor(
                                    out=osb[:, c0 + g, :],
                                    in0=o_ps[:, g * D : (g + 1) * D],
                                    in1=b2fsb[:, t * D : (t + 1) * D],
                                    op=mybir.AluOpType.add,
                                )

                for u in range(NU):
                    if u not in osbs:
                        continue
                    osb, C = osbs[u]
                    S = C * 128
                    s = nc.gpsimd.dma_scatter_add(
                        out_ap=outs[u][:],
                        in_ap=osb[:, 0:C, :],
                        idxs_ap=idxsb[:, scol[u, l] : scol[u, l] + S // 16],
                        num_idxs=S,
                        num_idxs_reg=creg_for(S),
                        elem_size=D,
                        single_packet=False,
                    )
                    add_dep_helper(s.ins, gathers[u].ins, sync=True, reason="war")
                    prev_scatter[u] = s

    from concourse.library_overlay import lower_extended_insts

    lower_extended_insts(nc)
    if split_waits:
        _cap_waits(nc)
    return nc


def assemble_output(results, sched, N, D, M):
    """results: list per core of dict out{u} -> np.ndarray."""
    NU = sched["NU"]
    unit_sizes = sched["unit_sizes"]
    parts = []
    for m in range(M):
        gs = [
            results[m][f"out{u}"][: unit_sizes[u] * N].reshape(unit_sizes[u], N, D)
            for u in range(NU)
        ]
        parts.append(np.concatenate(gs, axis=0))
    return np.concatenate(parts, axis=0)


def kernel(**inputs):
    import numpy as np

    root_embeddings = np.asarray(inputs["root_embeddings"], np.float32)
    W1 = np.asarray(inputs["W1"], np.float32)
    b1 = np.asarray(inputs["b1"], np.float32)
    W2 = np.asarray(inputs["W2"], np.float32)
    b2 = np.asarray(inputs["b2"], np.float32)
    idx = np.asarray(inputs["node_inputs_indices"], np.int32)
    types = np.asarray(inputs["node_types"], np.int32)

    B, N, R, D, T, M = B_, N_, R_, D_, T_, M_
    sched = build_schedule(idx, types, B, N, R, T, M)
    in_maps = build_inputs(root_embeddings, W1, b1, W2, b2, sched, N, R, D, T, M)
    nc = build_program(
        sched,
        N,
        R,
        D,
        T,
        zero_b1=not np.any(b1),
        zero_b2=not np.any(b2),
    )

    from concourse.bass_utils import run_bass_kernel_spmd

    res = run_bass_kernel_spmd(nc, in_maps, core_ids=list(range(M)))
    out = assemble_output(res.results, sched, N, D, M)
    return out.astype(np.float32)

